# revision 36
# baseline (speedup 1.0000x reference)
"""Multi-head self-attention (RoPE, causal) on 8 Trainium2 NeuronCores.

Sharding: core c -> (batch = c//2, head-group = c%2 of 8 heads).
Column-parallel wq/wk/wv, row-parallel wo. Each core emits a partial
out^T [f, s]; the host sums the two partials per batch and transposes.

Layouts (all chosen so no on-device transposes are needed):
  XT  [d, s]   (x transposed on host, bf16)
  Q^T/K^T [e, s] per head from matmul(lhsT=wT[d,e], rhs=XT[d,s])
  V   [s, e]   from matmul(lhsT=XT[d,s], rhs=wvT[d,e])
  S^T [j, i] = matmul(lhsT=K^T[e,j], rhs=Q^T[e,i])
  ctx^T [e, i] = matmul(lhsT=V[j,e], rhs=expS^T[j,i])
  out^T [f, s] = matmul(lhsT=woT[d,f], rhs=ctx^T[d,s])

All DRAM inputs/outputs are pre-tiled on the host into the exact
[128, N] blocks each DMA moves, so every DMA is a dense contiguous
copy. All matmul operands are bf16 (PSUM accumulation stays fp32);
softmax statistics and RoPE arithmetic stay fp32.

RoPE: head dims de-interleaved on host (even dims -> partitions 0..63,
odd -> 64..127 of each head's Q^T/K^T) by permuting wq/wk rows. Then
rot(x) = x*cc + (SP@x)*ss where SP is a signed permutation (matmul) and
cc/ss are host-precomputed fp32 cos/sin tables. The 1/sqrt(dk) scale is
applied via the Exp activation's scale field.

Softmax: no max-subtraction (scores are O(1)-scaled; fp32 exp is safe).
Causal masking by block-skipping + one 128x128 triangular mask on
diagonal blocks. Row sums via an all-ones [128,128] matmul (output rows
all equal the row sum, giving the partition broadcast for free);
normalization multiplies ctx^T by a fast DVE reciprocal of that tile.
"""

import numpy as np
import ml_dtypes

import concourse.bass as bass
import concourse.tile as tile
import concourse.mybir as mybir
from concourse import bacc, bass_utils

F32 = mybir.dt.float32
BF16 = mybir.dt.bfloat16

B = 4
S = 2048
D = 2048
NH = 16
DK = 128
NCORES = 8
HPC = 8            # heads per core
DLOC = HPC * DK    # 1024, local model dims per core
ST = S // 128      # 16 sequence 128-tiles
DT = D // 128      # 16 model-dim 128-tiles
NDT = DLOC // 128  # 8 local model-dim 128-tiles
IB = S // 512      # 4 i-blocks of 512
ROPE_THETA = 10000.0
SCALE = float(1.0 / np.sqrt(DK))

_cache = {}


def build_program():
    if "nc" in _cache:
        return _cache["nc"]

    nc = bacc.Bacc("TRN2", target_bir_lowering=False, debug=False,
                   num_devices=NCORES)

    xt = nc.dram_tensor("xt", [DT, 4, 128, 512], BF16, kind="ExternalInput").ap()
    wq = nc.dram_tensor("wq", [HPC, DT, 128, DK], BF16, kind="ExternalInput").ap()
    wk = nc.dram_tensor("wk", [HPC, DT, 128, DK], BF16, kind="ExternalInput").ap()
    wv = nc.dram_tensor("wv", [2, DT, 128, 512], BF16, kind="ExternalInput").ap()
    wo = nc.dram_tensor("wo", [NDT, 128, D], BF16, kind="ExternalInput").ap()
    cct = nc.dram_tensor("cct", [128, S], F32, kind="ExternalInput").ap()
    sst = nc.dram_tensor("sst", [128, S], F32, kind="ExternalInput").ap()
    sperm = nc.dram_tensor("sperm", [128, 128], BF16, kind="ExternalInput").ap()
    tri = nc.dram_tensor("tri", [128, 128], BF16, kind="ExternalInput").ap()
    out = nc.dram_tensor("out", [DT, IB, 128, 512], F32,
                         kind="ExternalOutput").ap()

    with tile.TileContext(nc) as tc:
        with (
            tc.tile_pool(name="dram", bufs=1, space="DRAM") as dram_pool,
            tc.tile_pool(name="ctx7", bufs=4) as ctx7_pool,
        ):
            ctx_dram = dram_pool.tile([HPC, IB, 128, 512], BF16)
            ctx7 = _attention_phase(nc, tc, xt, wq, wk, wv, cct, sst,
                                    sperm, tri, ctx_dram, ctx7_pool)
            _output_phase(nc, tc, wo, ctx_dram, out, ctx7)

    nc.compile()
    _cache["nc"] = nc
    return nc


def _attention_phase(nc, tc, xt, wq, wk, wv, cct, sst, sperm, tri, ctx_dram,
                     ctx7_pool):
    with (
        tc.tile_pool(name="xt", bufs=1) as xt_pool,
        tc.tile_pool(name="vsb", bufs=1) as v_pool,
        tc.tile_pool(name="tabs", bufs=1) as tab_pool,
        tc.tile_pool(name="wqk", bufs=2) as wqk_pool,
        tc.tile_pool(name="qkraw", bufs=2) as raw_pool,
        tc.tile_pool(name="rqk", bufs=2) as rqk_pool,
        tc.tile_pool(name="qk_ps", bufs=2, space="PSUM") as qk_ps_pool,
        tc.tile_pool(name="s_ps", bufs=2, space="PSUM") as s_ps_pool,
    ):
        # ---- resident loads (dense contiguous DMAs) ----
        def load_wqk(h):
            wq_sb = wqk_pool.tile([128, DT, DK], BF16, tag="wq")
            wk_sb = wqk_pool.tile([128, DT, DK], BF16, tag="wk")
            nc.sync.dma_start(wk_sb[:], wk[h].rearrange("d p k -> p d k",
                                                        p=128))
            nc.sync.dma_start(wq_sb[:], wq[h].rearrange("d p k -> p d k",
                                                        p=128))
            return wq_sb, wk_sb

        wqk0 = load_wqk(0)
        xt_sb = xt_pool.tile([128, DT, S], BF16)
        wv_sb = tab_pool.tile([128, DT, DLOC], BF16, tag="wv")
        cc_sb = tab_pool.tile([128, S], F32, tag="cct")
        ss_sb = tab_pool.tile([128, S], F32, tag="sst")
        sp_sb = tab_pool.tile([128, 128], BF16, tag="sperm")
        tri_sb = tab_pool.tile([128, 128], BF16, tag="tri")
        ones_sb = tab_pool.tile([128, 128], BF16, tag="ones")
        nc.gpsimd.memset(ones_sb[:], 1.0)
        nc.sync.dma_start(sp_sb[:], sperm)
        nc.sync.dma_start(tri_sb[:], tri)
        for ch in range(4):
            nc.sync.dma_start(
                xt_sb[:, :, ch * 512:(ch + 1) * 512],
                xt[:, ch].rearrange("d p c -> p d c", p=128))
            o = ch * 512
            nc.sync.dma_start(cc_sb[:, o:o + 512], cct[:, o:o + 512])
            nc.sync.dma_start(ss_sb[:, o:o + 512], sst[:, o:o + 512])
            if ch == 0:
                nc.sync.dma_start(
                    wv_sb[:, :, 0:512],
                    wv[0].rearrange("d p c -> p d c", p=128))
        nc.sync.dma_start(wv_sb[:, :, 512:1024],
                          wv[1].rearrange("d p c -> p d c", p=128))

        def proj_chunk(w_sb, r_t, ch):
            o = ch * 512
            ps = qk_ps_pool.tile([128, 512], F32, tag="qk_ps")
            for dt in range(DT):
                nc.tensor.matmul(
                    ps[:],
                    w_sb[:, dt, :],
                    xt_sb[:, dt, o:o + 512],
                    start=(dt == 0), stop=(dt == DT - 1),
                )
            raw = raw_pool.tile([128, 512], BF16, tag="qkraw")
            nc.scalar.copy(raw[:], ps[:])
            swp = s_ps_pool.tile([128, 512], F32, tag="s_ps")
            nc.tensor.matmul(swp[:], sp_sb[:], raw[:], start=True, stop=True)
            t2 = raw_pool.tile([128, 512], F32, tag="t2")
            nc.vector.tensor_mul(t2[:], swp[:], ss_sb[:, o:o + 512])
            t3 = raw_pool.tile([128, 512], F32, tag="t3")
            nc.vector.tensor_mul(t3[:], raw[:], cc_sb[:, o:o + 512])
            nc.vector.tensor_add(r_t[:, o:o + 512], t2[:], t3[:])

        def proj_rope(wq_sb, wk_sb):
            rq = rqk_pool.tile([128, S], BF16, tag="rq")
            rk = rqk_pool.tile([128, S], BF16, tag="rk")
            for ch in range(4):
                proj_chunk(wk_sb, rk, ch)
                proj_chunk(wq_sb, rq, ch)
            return rq, rk

        # head 0's projection is emitted per-chunk, interleaved with its
        # attention i-blocks (chunk ib is exactly what i-block ib consumes),
        # so DMA-paced chunks don't head-of-line-block ready attention work
        rq0 = rqk_pool.tile([128, S], BF16, tag="rq")
        rk0 = rqk_pool.tile([128, S], BF16, tag="rk")
        proj_chunk(wqk0[1], rk0, 0)
        proj_chunk(wqk0[0], rq0, 0)
        rqk0 = (rq0, rk0)

        # ---- V = x @ wv.T (emitted interleaved with head-0 attention) ----
        v_sb = v_pool.tile([128, ST, DLOC], BF16)

        def emit_v(st, g):
            v_ps = qk_ps_pool.tile([128, 512], F32, tag="qk_ps")
            for dt in range(DT):
                nc.tensor.matmul(
                    v_ps[:],
                    xt_sb[:, dt, st * 128:(st + 1) * 128],
                    wv_sb[:, dt, g * 512:(g + 1) * 512],
                    start=(dt == 0), stop=(dt == DT - 1),
                )
            nc.scalar.copy(v_sb[:, st, g * 512:(g + 1) * 512], v_ps[:])

        # ---- per-head attention (+ next head's projection interleaved) ----
        with (
            tc.tile_pool(name="exps", bufs=6) as exp_pool,
            tc.tile_pool(name="pair", bufs=2) as pair_pool,
            tc.tile_pool(name="small", bufs=2) as small_pool,
            tc.tile_pool(name="ctxsb", bufs=4) as ctx_sb_pool,
            tc.tile_pool(name="ctx_ps", bufs=2, space="PSUM") as ctx_ps_pool,
            tc.tile_pool(name="rs_ps", bufs=2, space="PSUM") as rs_ps_pool,
        ):
            ctx7 = []
            next_rqk = rqk0
            for h in range(HPC):
                rq, rk = next_rqk
                for ib in range(IB):
                    # emit the next head's projection + rope mid-attention so
                    # its DVE rope work drains during this head's i-blocks 2-3
                    if ib == 2 and h + 1 < HPC and h > 0:
                        next_rqk = proj_rope(*load_wqk(h + 1))
                    if h == 0 and ib == 3:
                        next_rqk = proj_rope(*load_wqk(1))
                    if h == 0:
                        if ib > 0:
                            proj_chunk(wqk0[1], rk, ib)
                            proj_chunk(wqk0[0], rq, ib)
                        # V tiles this i-block needs (g=0), just in time
                        for st in range(4 * ib, 4 * ib + 4):
                            emit_v(st, 0)
                    elif h == 1 and ib == 0:
                        for st in range(ST):
                            emit_v(st, 1)
                    i0 = ib * 512
                    ctx_ps = ctx_ps_pool.tile([128, 512], F32, tag="ctx_ps")
                    rs_ps = rs_ps_pool.tile([128, 512], F32, tag="rs_ps")
                    njt = 4 * ib + 4
                    es_prev = None
                    for jt in range(njt):
                        r = jt - 4 * ib  # >=0 on diagonal blocks
                        lo = 128 * r if r >= 0 else 0
                        s_ps = s_ps_pool.tile([128, 512], F32, tag="s_ps")
                        nc.tensor.matmul(
                            s_ps[:, lo:512],
                            rk[:, jt * 128:(jt + 1) * 128],
                            rq[:, i0 + lo:i0 + 512],
                            start=True, stop=True,
                        )
                        es = exp_pool.tile([128, 512], BF16, tag="exps")
                        nc.scalar.activation(es[:, lo:512], s_ps[:, lo:512],
                                             mybir.ActivationFunctionType.Exp,
                                             scale=SCALE)
                        if r >= 0:
                            nc.vector.tensor_mul(es[:, lo:lo + 128],
                                                 es[:, lo:lo + 128], tri_sb[:])
                        first = (jt == 0)
                        last = (jt == njt - 1)
                        # row sums: full (off-diagonal) tiles come in
                        # groups of 4; tree-sum each quad on DVE and quarter
                        # the RS matmuls
                        if r < 0 and jt % 2 == 0:
                            es_prev = es
                        elif r < 0 and jt % 4 == 1:
                            quad = pair_pool.tile([128, 512], BF16, tag="pair")
                            nc.vector.tensor_add(quad[:], es_prev[:], es[:])
                        elif r < 0:
                            pair = pair_pool.tile([128, 512], BF16, tag="pair")
                            nc.vector.tensor_add(pair[:], es_prev[:], es[:])
                            nc.vector.tensor_add(quad[:], quad[:], pair[:])
                            nc.tensor.matmul(
                                rs_ps[:],
                                ones_sb[:],
                                quad[:],
                                start=(jt == 3), stop=False,
                                skip_group_check=True,
                            )
                        else:
                            nc.tensor.matmul(
                                rs_ps[:, lo:512],
                                ones_sb[:],
                                es[:, lo:512],
                                start=first, stop=last, skip_group_check=True,
                            )
                        nc.tensor.matmul(
                            ctx_ps[:, lo:512],
                            v_sb[:, jt, h * DK:(h + 1) * DK],
                            es[:, lo:512],
                            start=first, stop=last, skip_group_check=True,
                        )
                    recip = small_pool.tile([128, 512], F32, tag="recip")
                    nc.vector.reciprocal_approx_fast(recip[:], rs_ps[:])
                    if h == HPC - 1:
                        ctx_sb = ctx7_pool.tile([128, 512], BF16, tag="c7")
                        ctx7.append(ctx_sb)
                    else:
                        ctx_sb = ctx_sb_pool.tile([128, 512], BF16,
                                                  tag="ctx_sb")
                    nc.vector.tensor_mul(ctx_sb[:], ctx_ps[:], recip[:])
                    if h != HPC - 1:
                        nc.sync.dma_start(ctx_dram[h, ib], ctx_sb[:])
            return ctx7


def _output_phase(nc, tc, wo, ctx_dram, out, ctx7):
    with (
        tc.tile_pool(name="wos", bufs=1) as wo_pool,
        tc.tile_pool(name="ctxin", bufs=2) as cin_pool,
        tc.tile_pool(name="outsb", bufs=3) as out_pool,
        tc.tile_pool(name="wo_ps", bufs=4, space="PSUM") as wo_ps_pool,
    ):
        wo_sb = wo_pool.tile([128, NDT, D], BF16)
        nc.sync.dma_start(wo_sb[:], wo.rearrange("d p f -> p d f", p=128))
        for sb4 in range(IB):
            cin = cin_pool.tile([128, NDT - 1, 512], BF16, tag="cin")
            nc.sync.dma_start(
                cin[:],
                ctx_dram[0:NDT - 1, sb4].rearrange("h p c -> p h c", p=128))
            # head 7's contribution comes from SBUF-resident ctx (no DRAM
            # round-trip on the tail); it is the last accumulation per group
            for ft in range(DT):
                ps = wo_ps_pool.tile([128, 512], F32, tag="wo_ps")
                for dt in range(NDT - 1):
                    nc.tensor.matmul(
                        ps[:],
                        wo_sb[:, dt, ft * 128:(ft + 1) * 128],
                        cin[:, dt, :],
                        start=(dt == 0), stop=False,
                    )
                nc.tensor.matmul(
                    ps[:],
                    wo_sb[:, NDT - 1, ft * 128:(ft + 1) * 128],
                    ctx7[sb4][:],
                    start=False, stop=True,
                )
                osb = out_pool.tile([128, 512], F32, tag="osb")
                nc.scalar.copy(osb[:], ps[:])
                nc.sync.dma_start(out[ft, sb4], osb[:])


def _tile2(a, p, q):
    """[R, C] -> [R//p, C//q, p, q] contiguous blocks."""
    R, C = a.shape
    return np.ascontiguousarray(
        a.reshape(R // p, p, C // q, q).transpose(0, 2, 1, 3))


def prepare_in_maps(x, wq, wk, wv, wo):
    """Build the 8 per-core input maps (host-side sharding + tables)."""
    x = np.asarray(x, dtype=np.float32)
    wq = np.asarray(wq, dtype=np.float32)
    wk = np.asarray(wk, dtype=np.float32)
    wv = np.asarray(wv, dtype=np.float32)
    wo = np.asarray(wo, dtype=np.float32)
    bf16 = ml_dtypes.bfloat16

    # RoPE tables (fp32, matching the reference's fp32 cos/sin)
    f = np.arange(0, DK, 2, dtype=np.float32) / DK          # 2f/d
    inv_freq = (ROPE_THETA ** (-f)).astype(np.float32)      # [64]
    ang = np.arange(S, dtype=np.float32)[:, None] * inv_freq[None, :]
    cos_t = np.cos(ang).T.astype(np.float32)                # [64, S]
    sin_t = np.sin(ang).T.astype(np.float32)
    cc = np.ascontiguousarray(np.vstack([cos_t, cos_t]))    # [128, S]
    ss = np.ascontiguousarray(np.vstack([sin_t, sin_t]))

    sperm = np.zeros((128, 128), dtype=np.float32)
    for m in range(64):
        sperm[m + 64, m] = -1.0       # out[m] = -in[m+64]
        sperm[m, m + 64] = 1.0        # out[m+64] = +in[m]
    sperm = sperm.astype(bf16)
    tri = np.tril(np.ones((128, 128), dtype=np.float32)).T  # tri[j,i]=1 if j<=i
    tri = np.ascontiguousarray(tri).astype(bf16)

    deint = np.concatenate([np.arange(0, DK, 2), np.arange(1, DK, 2)])
    in_maps = []
    for c in range(NCORES):
        bi, g = divmod(c, 2)
        heads = [g * HPC + h for h in range(HPC)]
        qk_rows = np.concatenate([hg * DK + deint for hg in heads])
        v_rows = np.arange(g * DLOC, (g + 1) * DLOC)

        xt_t = _tile2(x[bi].T.astype(bf16), 128, 512)            # [DT,4,128,512]
        # w*_prep [d, e_loc] -> [DT, HPC, 128, DK] -> [HPC, DT, 128, DK]
        wq_t = _tile2(wq[qk_rows, :].T.astype(bf16), 128, DK).transpose(1, 0, 2, 3)
        wk_t = _tile2(wk[qk_rows, :].T.astype(bf16), 128, DK).transpose(1, 0, 2, 3)
        # wv pre-tiled g-major: [2, DT, 128, 512]
        wv_t = _tile2(wv[v_rows, :].T.astype(bf16), 128, 512).transpose(1, 0, 2, 3)
        wo_t = _tile2(wo.T[v_rows, :].astype(bf16), 128, D)[:, 0]     # [NDT,128,D]
        in_maps.append({
            "xt": np.ascontiguousarray(xt_t),
            "wq": np.ascontiguousarray(wq_t),
            "wk": np.ascontiguousarray(wk_t),
            "wv": np.ascontiguousarray(wv_t),
            "wo": np.ascontiguousarray(wo_t),
            "cct": cc, "sst": ss,
            "sperm": sperm, "tri": tri,
        })
    return in_maps


def assemble(results):
    out = np.empty((B, S, D), dtype=np.float32)
    for bi in range(B):
        oT = results[2 * bi]["out"] + results[2 * bi + 1]["out"]
        # oT: [DT, IB, 128, 512] -> out^T [f, s]; out[b] = out^T.T
        oT = oT.transpose(0, 2, 1, 3).reshape(D, S)
        out[bi] = oT.T
    return out


def kernel(**inputs):
    nc = build_program()
    in_maps = prepare_in_maps(inputs["x"], inputs["wq"], inputs["wk"],
                              inputs["wv"], inputs["wo"])
    res = bass_utils.run_bass_kernel_spmd(nc, in_maps,
                                          core_ids=list(range(NCORES)))
    return assemble(res.results)
